# revision 3
# baseline (speedup 1.0000x reference)
"""Trainium2 Bass kernel for a 2-layer cross-encoder (CrossEncoder).

Model: B=2, NQ=NKV=2048, E=512, H=8 (d_head=64), MLP=2048, depth=2, fp32 I/O.

Sharding (8 cores, no collectives): core c handles batch b=c//4 and query
rows [qc*512, (qc+1)*512) with qc=c%4.  Each core computes the full KV
projections for its batch (duplicated across the 4 cores sharing a batch)
so every core produces its output slice independently.

Numerics: matmul operands are bf16 (fp32 PSUM accumulation everywhere);
residual stream, LayerNorm statistics and softmax normalization stay fp32.
LN gamma/beta are folded into the projection weights on the host.  The
softmax denominator comes free from a ones-column appended to V (rows of
softmax sum to one, and the un-normalized attn@V matmul also computes
sum(exp) in the extra column).  exp() needs no max-subtraction: scores are
O(1) here (weights scaled 0.02), so exp is well-conditioned.
"""

import numpy as np
import ml_dtypes

import concourse.bass as bass
import concourse.bacc as bacc
import concourse.mybir as mybir
import concourse.tile as tile
from concourse import bass_utils, masks
from contextlib import ExitStack

P = 128
E = 512
EC = E // P        # 4 chunks of the embedding dim
NQ = 512           # query rows per core
QC = NQ // P       # 4 query chunks
NKV = 2048
KC = NKV // P      # 16 key chunks of 128
KN = NKV // 512    # 4 key chunks of 512
H = 8
DH = 64
MLP = 2048
MC = MLP // P      # 16 mlp chunks of 128
L = 2
LN_EPS = 1e-5
F32 = mybir.dt.float32
BF16 = mybir.dt.bfloat16
AF = mybir.ActivationFunctionType
ALU = mybir.AluOpType
SCALE = (E // H) ** -0.5

_CACHE = {}


def _build():
    """Build the per-core Bass program (identical on all 8 cores)."""
    nc = bacc.Bacc("TRN2", target_bir_lowering=False, debug=False, num_devices=8)

    xq_d = nc.dram_tensor("xq", [NQ, E], F32, kind="ExternalInput").ap()
    xkv_d = nc.dram_tensor("xkv", [NKV, E], F32, kind="ExternalInput").ap()
    wd = []
    for l in range(L):
        wd.append({
            "wq": nc.dram_tensor(f"wq{l}", [P, EC * E], BF16, kind="ExternalInput").ap(),
            "wk": nc.dram_tensor(f"wk{l}", [P, EC * E], BF16, kind="ExternalInput").ap(),
            "wv": nc.dram_tensor(f"wv{l}", [P, EC * E], BF16, kind="ExternalInput").ap(),
            "wo": nc.dram_tensor(f"wo{l}", [P, EC * E], BF16, kind="ExternalInput").ap(),
            "w1": nc.dram_tensor(f"w1{l}", [P, EC * MLP], BF16, kind="ExternalInput").ap(),
            "w2": nc.dram_tensor(f"w2{l}", [P, MC * E], BF16, kind="ExternalInput").ap(),
            "bq": nc.dram_tensor(f"bq{l}", [P, EC], F32, kind="ExternalInput").ap(),
            "bk": nc.dram_tensor(f"bk{l}", [P, EC], F32, kind="ExternalInput").ap(),
            "b1": nc.dram_tensor(f"b1{l}", [P, MC], F32, kind="ExternalInput").ap(),
            "bo": nc.dram_tensor(f"bo{l}", [P, E], F32, kind="ExternalInput").ap(),
            "b2": nc.dram_tensor(f"b2{l}", [P, E], F32, kind="ExternalInput").ap(),
        })
    y_d = nc.dram_tensor("y", [NQ, E], F32, kind="ExternalOutput").ap()

    with tile.TileContext(nc) as tc, ExitStack() as ctx:
        const_pool = ctx.enter_context(tc.tile_pool(name="const", bufs=1))
        ident = const_pool.tile([P, P], BF16)
        masks.make_identity(nc, ident)

        stats_pool = ctx.enter_context(tc.tile_pool(name="stats", bufs=8))

        def ln_tile(x_t, out_pool, out_name):
            """LayerNorm core (x - mu) * rsqrt(var + eps), fp32 in, bf16 out."""
            bnst = stats_pool.tile([P, 6], F32, name="bnst")
            nc.vector.bn_stats(bnst[:], x_t)
            bnag = stats_pool.tile([P, 2], F32, name="bnag")
            nc.vector.bn_aggr(bnag[:], bnst[:])
            veps = stats_pool.tile([P, 1], F32, name="veps")
            nc.vector.tensor_scalar_add(veps[:], bnag[:, 1:2], LN_EPS)
            sq = stats_pool.tile([P, 1], F32, name="sq")
            nc.scalar.sqrt(sq[:], veps[:])
            rstd = stats_pool.tile([P, 1], F32, name="rstd")
            nc.vector.reciprocal(rstd[:], sq[:])
            h_t = out_pool.tile([P, E], BF16, name=out_name, bufs=3)
            nc.vector.tensor_scalar(
                h_t[:], x_t, bnag[:, 0:1], rstd[:], op0=ALU.subtract, op1=ALU.mult
            )
            return h_t

        # Residual stream: 4 fp32 tiles of [128, 512].
        xq_pool = ctx.enter_context(tc.tile_pool(name="xq", bufs=1))
        xq = []
        for i in range(QC):
            t = xq_pool.tile([P, E], F32, name=f"xq{i}", tag=f"xq{i}")
            nc.sync.dma_start(t[:], xq_d[i * P:(i + 1) * P, :])
            xq.append(t)

        # hkv^T: LN1-core of x_kv, transposed to [E, NKV].  ln1 g/b are folded
        # into the weights, so this is layer-independent: compute once.
        hkvT_pool = ctx.enter_context(tc.tile_pool(name="hkvT", bufs=1))
        hkvT = [
            hkvT_pool.tile([P, NKV], BF16, name=f"hkvT{e}", tag=f"hkvT{e}")
            for e in range(EC)
        ]

        # PSUM pools (8 banks total): pp 2 + ps_s 2x2 + ps_att 2 = 8.
        pp_pool = ctx.enter_context(tc.tile_pool(name="pp", bufs=2, space="PSUM"))
        ss_pool = ctx.enter_context(tc.tile_pool(name="ss", bufs=2, space="PSUM"))
        att_pool = ctx.enter_context(tc.tile_pool(name="attp", bufs=2, space="PSUM"))

        def transpose_block(dst, src_block, work_tag):
            """dst[128, 128] (slice of an SBUF tile) = src_block.T via PE."""
            pt = pp_pool.tile([P, E], F32, name="pp", tag="pp")
            ptb = pt[:].bitcast(BF16)[:, 0:P]
            nc.tensor.transpose(ptb, src_block, ident[:])
            nc.vector.tensor_copy(dst, ptb)

        with tc.tile_pool(name="xkv", bufs=3) as xkv_pool:
            for i in range(KC):
                xkv_t = xkv_pool.tile([P, E], F32, name="xkv_t", tag="xkv_t")
                nc.sync.dma_start(xkv_t[:], xkv_d[i * P:(i + 1) * P, :])
                hkv_t = ln_tile(xkv_t[:], xkv_pool, "hkv_t")
                for e in range(EC):
                    transpose_block(
                        hkvT[e][:, i * P:(i + 1) * P],
                        hkv_t[:, e * P:(e + 1) * P],
                        "tp",
                    )

        # Weight pools (bufs=2 -> next layer prefetches during current layer).
        wpool = ctx.enter_context(tc.tile_pool(name="w", bufs=2))
        # Work pools.
        work = ctx.enter_context(tc.tile_pool(name="work", bufs=1))
        big = ctx.enter_context(tc.tile_pool(name="big", bufs=1))
        ex_pool = ctx.enter_context(tc.tile_pool(name="ex", bufs=4))

        for l in range(L):
            w = wd[l]
            wq_sb = wpool.tile([P, EC * E], BF16, name="wq_sb", tag="wq")
            nc.sync.dma_start(wq_sb[:], w["wq"])
            wk_sb = wpool.tile([P, EC * E], BF16, name="wk_sb", tag="wk")
            nc.sync.dma_start(wk_sb[:], w["wk"])
            wv_sb = wpool.tile([P, EC * E], BF16, name="wv_sb", tag="wv")
            nc.sync.dma_start(wv_sb[:], w["wv"])
            wo_sb = wpool.tile([P, EC * E], BF16, name="wo_sb", tag="wo")
            nc.sync.dma_start(wo_sb[:], w["wo"])
            w1_sb = wpool.tile([P, EC * MLP], BF16, name="w1_sb", tag="w1", bufs=1)
            nc.sync.dma_start(w1_sb[:], w["w1"])
            w2_sb = wpool.tile([P, MC * E], BF16, name="w2_sb", tag="w2", bufs=1)
            nc.sync.dma_start(w2_sb[:], w["w2"])
            bq_sb = wpool.tile([P, EC], F32, name="bq_sb", tag="bq")
            nc.sync.dma_start(bq_sb[:], w["bq"])
            bk_sb = wpool.tile([P, EC], F32, name="bk_sb", tag="bk")
            nc.sync.dma_start(bk_sb[:], w["bk"])
            b1_sb = wpool.tile([P, MC], F32, name="b1_sb", tag="b1")
            nc.sync.dma_start(b1_sb[:], w["b1"])
            bo_sb = wpool.tile([P, E], F32, name="bo_sb", tag="bo")
            nc.sync.dma_start(bo_sb[:], w["bo"])
            b2_sb = wpool.tile([P, E], F32, name="b2_sb", tag="b2")
            nc.sync.dma_start(b2_sb[:], w["b2"])

            # ---- LN1(x_q) and transpose -> hqT [E, NQ] ----
            hqT = [
                work.tile([P, NQ], BF16, name=f"hqT{e}", tag=f"actT{e}")
                for e in range(EC)
            ]
            for qc in range(QC):
                hq_t = ln_tile(xq[qc][:], work, "hq_t")
                for e in range(EC):
                    transpose_block(
                        hqT[e][:, qc * P:(qc + 1) * P],
                        hq_t[:, e * P:(e + 1) * P],
                        "tp",
                    )

            # ---- q^T = wq^T @ hq^T + bq  [E, NQ] ----
            qT = [
                work.tile([P, NQ], BF16, name=f"qT{m}", tag=f"qT{m}")
                for m in range(EC)
            ]
            for m in range(EC):
                ps = pp_pool.tile([P, E], F32, name="pp", tag="pp")
                for kk in range(EC):
                    nc.tensor.matmul(
                        ps[:],
                        wq_sb[:, kk * E + m * P: kk * E + (m + 1) * P],
                        hqT[kk][:],
                        start=(kk == 0),
                        stop=(kk == EC - 1),
                    )
                nc.vector.tensor_scalar_add(qT[m][:], ps[:], bq_sb[:, m:m + 1])

            # ---- k^T = wk^T @ hkv^T + bk  [E, NKV] ----
            kT = [
                big.tile([P, NKV], BF16, name=f"kT{m}", tag=f"kT{m}")
                for m in range(EC)
            ]
            for m in range(EC):
                for n in range(KN):
                    ps = pp_pool.tile([P, E], F32, name="pp", tag="pp")
                    for kk in range(EC):
                        nc.tensor.matmul(
                            ps[:],
                            wk_sb[:, kk * E + m * P: kk * E + (m + 1) * P],
                            hkvT[kk][:, n * 512:(n + 1) * 512],
                            start=(kk == 0),
                            stop=(kk == EC - 1),
                        )
                    nc.vector.tensor_scalar_add(
                        kT[m][:, n * 512:(n + 1) * 512], ps[:], bk_sb[:, m:m + 1]
                    )

            # ---- v = hkv @ wv  [NKV, E], stored per key-chunk with a ones
            #      column per head: v_aug[m] is [128, H, DH+1] ----
            v_aug = [
                big.tile([P, H * (DH + 1)], BF16, name=f"vaug{m}", tag=f"vaug{m}")
                for m in range(KC)
            ]
            for m in range(KC):
                nc.vector.memset(v_aug[m][:], 1.0)
                ps = pp_pool.tile([P, E], F32, name="pp", tag="pp")
                for kk in range(EC):
                    nc.tensor.matmul(
                        ps[:],
                        hkvT[kk][:, m * P:(m + 1) * P],
                        wv_sb[:, kk * E:(kk + 1) * E],
                        start=(kk == 0),
                        stop=(kk == EC - 1),
                    )
                va = v_aug[m][:].rearrange("p (h d) -> p h d", h=H)
                nc.vector.tensor_copy(
                    va[:, :, 0:DH],
                    ps[:].rearrange("p (h d) -> p h d", h=H),
                )

            # ---- attention, head by head ----
            attnout = [
                work.tile([P, E], BF16, name=f"ao{qc}", tag=f"ao{qc}")
                for qc in range(QC)
            ]
            for h in range(H):
                fh, r0 = h // 2, (h % 2) * DH
                ps_att = att_pool.tile([P, QC, DH + 1], F32, name="ps_att", tag="att")
                for g in range(KC // 2):
                    ps_s = ss_pool.tile([P, 2, NQ], F32, name="ps_s", tag="ss")
                    for sub in range(2):
                        m = 2 * g + sub
                        nc.tensor.matmul(
                            ps_s[:, sub, :],
                            kT[fh][r0:r0 + DH, m * P:(m + 1) * P],
                            qT[fh][r0:r0 + DH, :],
                            start=True,
                            stop=True,
                        )
                    ex = ex_pool.tile([P, 2, NQ], BF16, name="ex", tag="ex")
                    nc.scalar.activation(ex[:], ps_s[:], AF.Exp, scale=SCALE)
                    for sub in range(2):
                        m = 2 * g + sub
                        va = v_aug[m][:].rearrange("p (h d) -> p h d", h=H)
                        for qc in range(QC):
                            nc.tensor.matmul(
                                ps_att[:, qc, :],
                                ex[:, sub, qc * P:(qc + 1) * P],
                                va[:, h, :],
                                start=(m == 0),
                                stop=(m == KC - 1),
                            )
                for qc in range(QC):
                    rec = stats_pool.tile([P, 1], F32, name="rec")
                    nc.vector.reciprocal(rec[:], ps_att[:, qc, DH:DH + 1])
                    nc.vector.tensor_scalar_mul(
                        attnout[qc][:, h * DH:(h + 1) * DH],
                        ps_att[:, qc, 0:DH],
                        rec[:],
                    )

            # ---- attnout^T, out-proj, residual ----
            aoT = [
                work.tile([P, NQ], BF16, name=f"aoT{e}", tag=f"aoT{e}")
                for e in range(EC)
            ]
            for qc in range(QC):
                for e in range(EC):
                    transpose_block(
                        aoT[e][:, qc * P:(qc + 1) * P],
                        attnout[qc][:, e * P:(e + 1) * P],
                        "tp",
                    )
            for qc in range(QC):
                ps = pp_pool.tile([P, E], F32, name="pp", tag="pp")
                for kk in range(EC):
                    nc.tensor.matmul(
                        ps[:],
                        aoT[kk][:, qc * P:(qc + 1) * P],
                        wo_sb[:, kk * E:(kk + 1) * E],
                        start=(kk == 0),
                        stop=(kk == EC - 1),
                    )
                nc.vector.tensor_add(ps[:], ps[:], bo_sb[:])
                nc.vector.tensor_add(xq[qc][:], xq[qc][:], ps[:])

            # ---- LN2 + transpose -> h2T ----
            h2T = [
                work.tile([P, NQ], BF16, name=f"h2T{e}", tag=f"actT{e}")
                for e in range(EC)
            ]
            for qc in range(QC):
                h2_t = ln_tile(xq[qc][:], work, "hq_t")
                for e in range(EC):
                    transpose_block(
                        h2T[e][:, qc * P:(qc + 1) * P],
                        h2_t[:, e * P:(e + 1) * P],
                        "tp",
                    )

            # ---- FFN1: g^T = gelu(w1^T @ h2^T + b1)  [MLP, NQ] ----
            gT = [
                big.tile([P, NQ], BF16, name=f"gT{m}", tag=f"gT{m}")
                for m in range(MC)
            ]
            for m in range(MC):
                ps = pp_pool.tile([P, E], F32, name="pp", tag="pp")
                for kk in range(EC):
                    nc.tensor.matmul(
                        ps[:],
                        w1_sb[:, kk * MLP + m * P: kk * MLP + (m + 1) * P],
                        h2T[kk][:],
                        start=(kk == 0),
                        stop=(kk == EC - 1),
                    )
                nc.scalar.activation(gT[m][:], ps[:], AF.Gelu, bias=b1_sb[:, m:m + 1])

            # ---- FFN2 + residual ----
            for qc in range(QC):
                ps = pp_pool.tile([P, E], F32, name="pp", tag="pp")
                for m in range(MC):
                    nc.tensor.matmul(
                        ps[:],
                        gT[m][:, qc * P:(qc + 1) * P],
                        w2_sb[:, m * E:(m + 1) * E],
                        start=(m == 0),
                        stop=(m == MC - 1),
                    )
                nc.vector.tensor_add(ps[:], ps[:], b2_sb[:])
                nc.vector.tensor_add(xq[qc][:], xq[qc][:], ps[:])

        for qc in range(QC):
            nc.sync.dma_start(y_d[qc * P:(qc + 1) * P, :], xq[qc][:])

    nc.compile()
    return nc


def get_nc():
    if "nc" not in _CACHE:
        _CACHE["nc"] = _build()
    return _CACHE["nc"]


def _rearr(w, k):
    """[k*128, C] row-major -> [128, k*C] with free layout (chunk, col)."""
    c = w.shape[1]
    return np.ascontiguousarray(
        w.reshape(k, P, c).transpose(1, 0, 2).reshape(P, k * c)
    )


def _cols(v):
    """[k*128] -> [128, k]: column m holds v[m*128:(m+1)*128]."""
    k = v.shape[0] // P
    return np.ascontiguousarray(v.reshape(k, P).T)


def _bf16(a):
    return np.asarray(a, dtype=np.float32).astype(ml_dtypes.bfloat16)


def kernel(**inputs) -> np.ndarray:
    x_q = np.asarray(inputs["x_q"], np.float32)
    x_kv = np.asarray(inputs["x_kv"], np.float32)
    wq = np.asarray(inputs["wq"], np.float32)
    wkv = np.asarray(inputs["wkv"], np.float32)
    wo = np.asarray(inputs["wo"], np.float32)
    bo = np.asarray(inputs["bo"], np.float32)
    w1 = np.asarray(inputs["w1"], np.float32)
    b1 = np.asarray(inputs["b1"], np.float32)
    w2 = np.asarray(inputs["w2"], np.float32)
    b2 = np.asarray(inputs["b2"], np.float32)
    ln1_g = np.asarray(inputs["ln1_g"], np.float32)
    ln1_b = np.asarray(inputs["ln1_b"], np.float32)
    ln2_g = np.asarray(inputs["ln2_g"], np.float32)
    ln2_b = np.asarray(inputs["ln2_b"], np.float32)

    # Host-side folding of LN affine params into the projection weights.
    shared = {}
    for l in range(L):
        wk_f = wkv[l][:, :E]
        wv_f = wkv[l][:, E:]
        wq_eff = ln1_g[l][:, None] * wq[l]
        wk_eff = ln1_g[l][:, None] * wk_f
        wv_eff = ln1_g[l][:, None] * wv_f
        bq_eff = ln1_b[l] @ wq[l]
        bk_eff = ln1_b[l] @ wk_f
        bv_eff = ln1_b[l] @ wv_f
        bo_eff = bo[l] + bv_eff @ wo[l]
        w1_eff = ln2_g[l][:, None] * w1[l]
        b1_eff = ln2_b[l] @ w1[l] + b1[l]
        shared.update({
            f"wq{l}": _rearr(_bf16(wq_eff), EC),
            f"wk{l}": _rearr(_bf16(wk_eff), EC),
            f"wv{l}": _rearr(_bf16(wv_eff), EC),
            f"wo{l}": _rearr(_bf16(wo[l]), EC),
            f"w1{l}": _rearr(_bf16(w1_eff), EC),
            f"w2{l}": _rearr(_bf16(w2[l]), MC),
            f"bq{l}": _cols(bq_eff),
            f"bk{l}": _cols(bk_eff),
            f"b1{l}": _cols(b1_eff),
            f"bo{l}": np.ascontiguousarray(np.broadcast_to(bo_eff, (P, E))),
            f"b2{l}": np.ascontiguousarray(np.broadcast_to(b2[l], (P, E))),
        })

    in_maps = []
    for c in range(8):
        b, qc = c // 4, c % 4
        m = dict(shared)
        m["xq"] = np.ascontiguousarray(x_q[b, qc * NQ:(qc + 1) * NQ, :])
        m["xkv"] = np.ascontiguousarray(x_kv[b])
        in_maps.append(m)

    nc = get_nc()
    res = bass_utils.run_bass_kernel_spmd(nc, in_maps, core_ids=list(range(8)))

    out = np.empty((2, 2048, E), np.float32)
    for c in range(8):
        b, qc = c // 4, c % 4
        out[b, qc * NQ:(qc + 1) * NQ, :] = res.results[c]["y"]
    return out
